# revision 22
# baseline (speedup 1.0000x reference)
"""DRXNet forward + dVii VJP on 8 Trainium2 NeuronCores (Bass/Tile).

Data-parallel over crystals: 256 crystals (2048 nodes, 16384 edges) per core.
Feature-major layout in SBUF; edge gathers are strided/broadcast matmul rhs
views (fully-connected 8-node crystals = regular patterns). Gate softmax runs
in a stacked [128, 512] layout (4 column-packed 32-row replicated blocks) so
segment ops are free-dim tensor_reduces on 128 lanes. Softmax is computed
without the segment-max shift (shift-invariant; the 1e-10 eps term changes by
exp(gmax), a ~1e-9 relative effect) and the gate output bias cancels in the
normalization, so both are dropped. Only Vii needs a VJP: the gradient flows
through the small voltage-branch MLPs only, computed analytically.
"""
import sys
import contextlib
import numpy as np

sys.path.insert(0, "/opt/trn_rl_repo")

C = 2048
NPC = 8
NCORES = 8
CL = C // NCORES      # 256 crystals/core
NL = CL * NPC         # 2048 nodes/core
EL = CL * NPC * NPC   # 16384 edges/core
NG = 8                # edge groups of 2048
NS = 4                # 512-edge subtiles per group
EPS = 1e-10

_CACHE = {}


def _a(x):
    return np.ascontiguousarray(np.asarray(x, dtype=np.float32))


def _pad_cols(w, n):
    o = np.zeros((w.shape[0], n), np.float32)
    o[:, :w.shape[1]] = w
    return o


def _pad_rows(w, n):
    o = np.zeros((n, w.shape[1]), np.float32)
    o[:w.shape[0]] = w
    return o


TAIL_NAMES = [
    ("rate_fc1", [1, 32]), ("rate_b1", [32, 1]), ("rate_fc2", [32, 32]), ("rate_b2", [32, 1]),
    ("cyc_fc1", [1, 32]), ("cyc_b1", [32, 1]), ("cyc_fc2", [32, 32]), ("cyc_b2", [32, 1]),
    ("er_fc1", [32, 32]), ("er_fc2", [16, 32]), ("er_fcb", [32, 1]),
    ("er_g1", [32, 32]), ("er_g2", [16, 32]), ("er_gb", [32, 1]),
    ("ec_fc1", [32, 32]), ("ec_fc2", [16, 32]), ("ec_fcb", [32, 1]),
    ("ec_g1", [32, 32]), ("ec_g2", [16, 32]), ("ec_gb", [32, 1]),
    ("dNw", [1, 32]),
    ("v_fc1", [3, 64]), ("v_b1", [64, 1]), ("v_fc2", [64, 32]), ("v_b2", [32, 1]),
    ("av_fc", [32, 32]), ("av_b", [32, 1]),
    ("q_fc1", [32, 32]), ("q_b1", [32, 1]), ("q_fc2", [32, 1]), ("q_b2", [1, 1]),
    ("v_fc1T", [64, 3]), ("v_fc2T", [32, 64]), ("av_fcT", [32, 32]),
    ("q_fc1T", [32, 32]), ("q_fc2T", [1, 32]),
]


def _build():
    import concourse.bass as bass
    from concourse import bacc, mybir
    import concourse.tile as tile

    f32 = mybir.dt.float32
    f32r = mybir.dt.float32r
    AF = mybir.ActivationFunctionType
    ALU = mybir.AluOpType

    nc = bacc.Bacc(trn_type="TRN2")
    EIN = dict(kind="ExternalInput")

    d_efT = nc.dram_tensor("efT", [200, NL], f32r, **EIN)
    d_ew = nc.dram_tensor("ew_row", [1, NL], f32r, **EIN)
    d_wnbr = nc.dram_tensor("wnbr_st", [128, NG, 512], f32, **EIN)
    d_wcry = nc.dram_tensor("wcry_st", [128, 512], f32, **EIN)
    d_u0 = nc.dram_tensor("u0T", [3, CL], f32, **EIN)
    d_rate = nc.dram_tensor("rateT", [1, CL], f32, **EIN)
    d_cyc = nc.dram_tensor("cycT", [1, CL], f32, **EIN)
    d_wemb0 = nc.dram_tensor("wemb0", [128, 31], f32r, **EIN)
    d_wemb1 = nc.dram_tensor("wemb1", [72, 31], f32r, **EIN)
    d_bemb = nc.dram_tensor("bemb", [31, 1], f32, **EIN)

    d_W1, d_W1n, d_b1, d_w2g, d_W2m, d_b2m = {}, {}, {}, {}, {}, {}
    for l in range(3):
        for h in range(3):
            d_W1[l, h] = nc.dram_tensor(f"W1s_{l}{h}", [32, 512], f32r, **EIN)
            d_W1n[l, h] = nc.dram_tensor(f"W1n_{l}{h}", [32, 512], f32r, **EIN)
            d_b1[l, h] = nc.dram_tensor(f"b1_{l}{h}", [128, 4], f32, **EIN)
            d_w2g[l, h] = nc.dram_tensor(f"w2g_{l}{h}", [128, 2, 32], f32r, **EIN)
            d_W2m[l, h] = nc.dram_tensor(f"W2m_{l}{h}", [128, 2, 32], f32r, **EIN)
            d_b2m[l, h] = nc.dram_tensor(f"b2m_{l}{h}", [128, 1], f32, **EIN)
    d_W1c, d_b1c, d_w2gc, d_W2mc, d_b2mc = {}, {}, {}, {}, {}
    for h in range(3):
        d_W1c[h] = nc.dram_tensor(f"W1c_{h}", [32, 512], f32r, **EIN)
        d_b1c[h] = nc.dram_tensor(f"b1c_{h}", [128, 4], f32, **EIN)
        d_w2gc[h] = nc.dram_tensor(f"w2gc_{h}", [128, 2, 32], f32r, **EIN)
        d_W2mc[h] = nc.dram_tensor(f"W2mc_{h}", [128, 2, 32], f32r, **EIN)
        d_b2mc[h] = nc.dram_tensor(f"b2mc_{h}", [128, 1], f32, **EIN)
    d_tail = {n: nc.dram_tensor(n, s, f32, **EIN) for n, s in TAIL_NAMES}
    d_outq = nc.dram_tensor("outq", [1, CL], f32, kind="ExternalOutput")
    d_outg = nc.dram_tensor("outg", [1, CL], f32, kind="ExternalOutput")
    DBG = bool(int(__import__("os").environ.get("KDBG", "0")))
    d_dbg = {}
    if DBG:
        for nm, shp in [("fea0", [32, NL]), ("fea1", [32, NL]), ("fea3", [32, NL]),
                        ("crys", [32, CL]), ("condc", [32, CL]), ("g2d", [1, CL]),
                        ("gate00", [128, 512]), ("msg00", [128, 512]),
                        ("eww00", [128, 512]), ("pacc0", [128, 64])]:
            d_dbg[nm] = nc.dram_tensor(nm, shp, f32, kind="ExternalOutput")

    with tile.TileContext(nc) as tc, contextlib.ExitStack() as ctx:
        persist = ctx.enter_context(tc.tile_pool(name="persist", bufs=1))
        wpool = ctx.enter_context(tc.tile_pool(name="wpool", bufs=1))
        hidp = ctx.enter_context(tc.tile_pool(name="hidp", bufs=3))
        gwork = ctx.enter_context(tc.tile_pool(name="gwork", bufs=3))
        small = ctx.enter_context(tc.tile_pool(name="small", bufs=1))
        pp = ctx.enter_context(tc.tile_pool(name="pp", bufs=2, space="PSUM"))
        pg = ctx.enter_context(tc.tile_pool(name="pg", bufs=2, space="PSUM"))

        def ld(pool, name, shape, dram, dt=f32):
            t = pool.tile(shape, dt, tag=name)
            nc.sync.dma_start(t[:], dram[:])
            return t

        consts = wpool.tile([128, 4], f32, tag="consts")
        nc.vector.memset(consts[:, 0:1], 0.0)
        nc.vector.memset(consts[:, 1:2], float(EPS))
        nc.vector.memset(consts[:, 2:3], -1.0)
        nc.vector.memset(consts[:, 3:4], 1.0)

        def softplus(dst, srcap, p, ttag):
            # stable: max(x,0) + ln(1 + exp(-|x|))
            t = small.tile([p, CL], f32, tag=ttag, name=ttag)
            nc.scalar.activation(t[:], srcap, AF.Abs, bias=cb(p, 0), scale=1.0)
            nc.scalar.activation(t[:], t[:], AF.Exp, bias=cb(p, 0), scale=-1.0)
            nc.scalar.activation(t[:], t[:], AF.Ln, bias=cb(p, 3), scale=1.0)
            t2 = small.tile([p, CL], f32, tag=ttag + "b", name=ttag + "b")
            nc.vector.tensor_scalar(out=t2[:], in0=srcap, scalar1=0.0,
                                    scalar2=None, op0=ALU.max)
            nc.vector.tensor_tensor(dst, t[:], t2[:], ALU.add)

        def cb(p, i):
            return consts[0:p, i:i + 1]

        wnbr = ld(wpool, "wnbr", [128, NG, 512], d_wnbr)
        wcry = ld(wpool, "wcry", [128, 512], d_wcry)
        W1, W1n, b1, w2g, W2m, b2m = {}, {}, {}, {}, {}, {}
        for l in range(3):
            for h in range(3):
                W1[l, h] = ld(wpool, f"W1s_{l}{h}", [32, 512], d_W1[l, h], f32r)
                W1n[l, h] = ld(wpool, f"W1n_{l}{h}", [32, 512], d_W1n[l, h], f32r)
                b1[l, h] = ld(wpool, f"b1_{l}{h}", [128, 4], d_b1[l, h])
                w2g[l, h] = ld(wpool, f"w2g_{l}{h}", [128, 2, 32], d_w2g[l, h], f32r)
                W2m[l, h] = ld(wpool, f"W2m_{l}{h}", [128, 2, 32], d_W2m[l, h], f32r)
                b2m[l, h] = ld(wpool, f"b2m_{l}{h}", [128, 1], d_b2m[l, h])
        W1c, b1c, w2gc, W2mc, b2mc = {}, {}, {}, {}, {}
        for h in range(3):
            W1c[h] = ld(wpool, f"W1c_{h}", [32, 512], d_W1c[h], f32r)
            b1c[h] = ld(wpool, f"b1c_{h}", [128, 4], d_b1c[h])
            w2gc[h] = ld(wpool, f"w2gc_{h}", [128, 2, 32], d_w2gc[h], f32r)
            W2mc[h] = ld(wpool, f"W2mc_{h}", [128, 2, 32], d_W2mc[h], f32r)
            b2mc[h] = ld(wpool, f"b2mc_{h}", [128, 1], d_b2mc[h])
        tw = {n: ld(wpool, f"tw_{n}", s, d_tail[n], f32) for n, s in TAIL_NAMES}

        feaT = persist.tile([32, NL], f32r, tag="fea_0")
        with tc.tile_pool(name="eph", bufs=1) as eph:
            ef0 = eph.tile([128, NL], f32r, tag="ef0")
            nc.sync.dma_start(ef0[:], d_efT[0:128, :])
            ef1 = eph.tile([72, NL], f32r, tag="ef1")
            nc.sync.dma_start(ef1[:], d_efT[128:200, :])
            wemb0 = ld(eph, "wemb0", [128, 31], d_wemb0, f32r)
            wemb1 = ld(eph, "wemb1", [72, 31], d_wemb1, f32r)
            bemb = ld(eph, "bemb", [31, 1], d_bemb)

            nc.sync.dma_start(feaT[31:32, :], d_ew[:])
            for j in range(NL // 512):
                ps = pp.tile([31, 512], f32, tag="fc1_0")
                nc.tensor.matmul(ps[:], wemb0[:], ef0[:, bass.ts(j, 512)],
                                 start=True, stop=False)
                nc.tensor.matmul(ps[:], wemb1[:], ef1[:, bass.ts(j, 512)],
                                 start=False, stop=True)
                nc.scalar.activation(feaT[0:31, bass.ts(j, 512)], ps[:],
                                     AF.Identity, bias=bemb[:], scale=1.0)

        def gather_views(src, g, c, strip_s, strip_n):
            n0 = g * 256 + c * 64
            pitch = src.tensor.shape[-1]
            base = src[:].offset + n0
            sv = bass.AP(tensor=src.tensor, offset=base + strip_s * 32 * pitch,
                         ap=[[pitch, 32], [8, 8], [1, 8], [0, 8]])
            nv = bass.AP(tensor=src.tensor, offset=base + strip_n * 32 * pitch,
                         ap=[[pitch, 32], [8, 8], [0, 8], [1, 8]])
            return sv, nv

        dbg_state = {"first": True}

        def wap_accum(src, W1t, W1nt, b1t, w2gt, W2mt, b2mt, wn_g, g, cry_mode,
                      pool_acc, first_head):
            """One WAP head over edge-group g; accumulates pooled/3 into
            pool_acc [128, 64] (stacked node layout)."""
            gate_ps = pg.tile([128, 512], f32, tag="gate_ps")
            msg_ps = pg.tile([128, 512], f32, tag="msg_ps")
            for c in range(NS):
                fc1 = [pp.tile([128, 512], f32, tag=f"fc1_{k}", name=f"fc1_{k}") for k in range(4)]
                for k in range(4):
                    if cry_mode:
                        rhs = src[0:32, bass.ds(c * 512, 512)]
                        nc.tensor.matmul(fc1[k][:], W1t[:, bass.ts(k, 128)],
                                         rhs, start=True, stop=True)
                    else:
                        sv, nv = gather_views(src, g, c)
                        nc.tensor.matmul(fc1[k][:], W1t[:, bass.ts(k, 128)],
                                         sv, start=True, stop=False)
                        nc.tensor.matmul(fc1[k][:], W1nt[:, bass.ts(k, 128)],
                                         nv, start=False, stop=True)
                hs = hidp.tile([128, 4, 512], f32r, tag="hid_sb")
                for k in range(4):
                    if k % 2 == 0:
                        nc.vector.tensor_scalar(
                            out=hs[:, k, :], in0=fc1[k][:],
                            scalar1=b1t[:, k:k + 1], scalar2=0.0,
                            op0=ALU.add, op1=ALU.max)
                    else:
                        nc.scalar.activation(hs[:, k, :], fc1[k][:], AF.Relu,
                                             bias=b1t[:, k:k + 1], scale=1.0)
                for k in range(2):
                    nc.tensor.matmul(gate_ps[bass.ts(c, 32), :],
                                     w2gt[:, k, :], hs[:, k, :],
                                     start=(k == 0), stop=(k == 1),
                                     tile_position=(0, c * 32))
                for k in range(2):
                    nc.tensor.matmul(msg_ps[bass.ts(c, 32), :],
                                     W2mt[:, k, :], hs[:, 2 + k, :],
                                     start=(k == 0), stop=(k == 1),
                                     tile_position=(0, c * 32))
            dbg_first = DBG and dbg_state["first"]
            if dbg_first:
                dbg_state["first"] = False
                gtmp = gwork.tile([128, 512], f32, tag="gtmp")
                nc.vector.tensor_copy(gtmp[:], gate_ps[:])
                nc.sync.dma_start(d_dbg["gate00"][:], gtmp[:])
                mtmp = gwork.tile([128, 512], f32, tag="mtmp")
                nc.vector.tensor_copy(mtmp[:], msg_ps[:])
                nc.sync.dma_start(d_dbg["msg00"][:], mtmp[:])
            ew_t = gwork.tile([128, 512], f32, tag="ew")
            nc.scalar.activation(ew_t[:], gate_ps[:], AF.Exp, bias=cb(128, 0), scale=1.0)
            eww = gwork.tile([128, 512], f32, tag="eww")
            nc.vector.tensor_tensor(eww[:], ew_t[:], wn_g, ALU.mult)
            gs = small.tile([128, 64], f32, tag="gs")
            nc.vector.tensor_reduce(
                out=gs[:], in_=eww[:].rearrange("p (n e) -> p n e", e=8),
                axis=mybir.AxisListType.X, op=ALU.add)
            rcp = small.tile([128, 64], f32, tag="rcp")
            nc.scalar.activation(rcp[:], gs[:], AF.Identity, bias=cb(128, 1),
                                 scale=1.0)
            nc.vector.reciprocal(out=rcp[:], in_=rcp[:])
            gm = gwork.tile([128, 512], f32, tag="gm")
            nc.vector.tensor_tensor(gm[:], msg_ps[:], eww[:], ALU.mult)
            pun = small.tile([128, 64], f32, tag="pun")
            nc.vector.tensor_reduce(
                out=pun[:], in_=gm[:].rearrange("p (n e) -> p n e", e=8),
                axis=mybir.AxisListType.X, op=ALU.add)
            if dbg_first:
                nc.sync.dma_start(d_dbg["eww00"][:], eww[:])
            pn = small.tile([128, 64], f32, tag="pn")
            nc.vector.tensor_tensor(pn[:], pun[:], rcp[:], ALU.mult)
            if first_head:
                nc.vector.tensor_scalar(out=pool_acc[:], in0=pn[:],
                                        scalar1=b2mt[:], scalar2=1.0 / 3.0,
                                        op0=ALU.add, op1=ALU.mult)
            else:
                tmp = small.tile([128, 64], f32, tag="pn2")
                nc.vector.tensor_scalar(out=tmp[:], in0=pn[:],
                                        scalar1=b2mt[:], scalar2=1.0 / 3.0,
                                        op0=ALU.add, op1=ALU.mult)
                nc.vector.tensor_tensor(pool_acc[:], pool_acc[:], tmp[:], ALU.add)

        def destack_dma(dst_tile, col0, src_tile):
            # src [128, 64] stacked (rows 32c+f) -> dst[f, col0 + c*64 + j]
            for c in range(4):
                nc.sync.dma_start(
                    dst_tile[0:32, bass.ds(col0 + c * 64, 64)],
                    src_tile[bass.ts(c, 32), :])

        if DBG:
            nc.sync.dma_start(d_dbg["fea0"][:], feaT[0:32, :].bitcast(f32))
        cur = feaT
        for l in range(3):
            nxt = persist.tile([32, NL], f32r, tag=f"fea_{l + 1}")
            for g in range(NG):
                pool_acc = gwork.tile([128, 64], f32r, tag="pool_acc")
                for h in range(3):
                    wap_accum(cur, W1[l, h], W1n[l, h], b1[l, h], w2g[l, h],
                              W2m[l, h], b2m[l, h], wnbr[:, g, :], g, False,
                              pool_acc, h == 0)
                if DBG and l == 0 and g == 0:
                    ptmp = gwork.tile([128, 64], f32, tag="ptmp")
                    nc.vector.tensor_copy(ptmp[:], pool_acc[:])
                    nc.sync.dma_start(d_dbg["pacc0"][:], ptmp[:])
                destack_dma(nxt, g * 256, pool_acc)
            nc.vector.tensor_tensor(nxt[:], nxt[:], cur[0:32, :], ALU.add)
            if DBG and l == 0:
                nc.sync.dma_start(d_dbg["fea1"][:], nxt[0:32, :].bitcast(f32))
            if DBG and l == 2:
                nc.sync.dma_start(d_dbg["fea3"][:], nxt[0:32, :].bitcast(f32))
            cur = nxt

        # cry_pool -> crys at rows 0-31 of cat1 [48, CL]
        cat1 = persist.tile([32, CL], f32, tag="cat1")
        crys_st = gwork.tile([128, 64], f32, tag="crys_st")
        for h in range(3):
            wap_accum(cur, W1c[h], None, b1c[h], w2gc[h], W2mc[h], b2mc[h],
                      wcry[:], 0, True, crys_st, h == 0)
        destack_dma(cat1, 0, crys_st)
        if DBG:
            nc.sync.dma_start(d_dbg["crys"][:], cat1[:].bitcast(f32))

        rateT = small.tile([1, CL], f32, tag="rateT")
        nc.sync.dma_start(rateT[:], d_rate[:])
        cycT = small.tile([1, CL], f32, tag="cycT")
        nc.sync.dma_start(cycT[:], d_cyc[:])
        u0 = small.tile([3, CL], f32, tag="u0")
        nc.sync.dma_start(u0[:], d_u0[:])

        def two_mlp(inp, fc1n, b1n, fc2n, b2n):
            ps1 = pp.tile([32, CL], f32, tag="fc1_0")
            nc.tensor.matmul(ps1[:], tw[fc1n][:], inp, start=True, stop=True)
            hh = small.tile([32, CL], f32, tag=f"h_{fc1n}", name=f"h_{fc1n}")
            nc.scalar.activation(hh[:], ps1[:], AF.Relu, bias=tw[b1n][:], scale=1.0)
            ps2 = pp.tile([32, CL], f32, tag="fc1_0")
            nc.tensor.matmul(ps2[:], tw[fc2n][:], hh[:], start=True, stop=True)
            o16 = small.tile([16, CL], f32, tag=f"o_{fc1n}", name=f"o_{fc1n}")
            nc.scalar.activation(o16[:], ps2[0:16, :], AF.Identity,
                                 bias=tw[b2n][0:16, :], scale=1.0)
            return o16

        rate_fea = two_mlp(rateT[:], "rate_fc1", "rate_b1", "rate_fc2", "rate_b2")

        def gate_mod(catA, catB, f1, f2, fcbn, g1n, g2n, gbn, otag):
            psg = pp.tile([32, CL], f32, tag="fc1_0")
            nc.tensor.matmul(psg[:], tw[g1n][:], catA, start=True, stop=False)
            nc.tensor.matmul(psg[:], tw[g2n][:], catB, start=False, stop=True)
            attn = small.tile([32, CL], f32, tag=f"at_{otag}", name=f"at_{otag}")
            nc.scalar.activation(attn[:], psg[:], AF.Sigmoid, bias=tw[gbn][:],
                                 scale=1.0)
            psf = pp.tile([32, CL], f32, tag="fc1_0")
            nc.tensor.matmul(psf[:], tw[f1][:], catA, start=True, stop=False)
            nc.tensor.matmul(psf[:], tw[f2][:], catB, start=False, stop=True)
            fcv = small.tile([32, CL], f32, tag=f"fc_{otag}", name=f"fc_{otag}")
            nc.scalar.activation(fcv[:], psf[:], AF.Relu, bias=tw[fcbn][:],
                                 scale=1.0)
            o = small.tile([32, CL], f32, tag=f"go_{otag}", name=f"go_{otag}")
            nc.vector.tensor_tensor(o[:], fcv[:], attn[:], ALU.mult)
            return o

        cr = gate_mod(cat1[0:32, :], rate_fea[:], "er_fc1", "er_fc2", "er_fcb",
                      "er_g1", "er_g2", "er_gb", "er")
        cond_rate = small.tile([32, CL], f32, tag="cond_rate")
        nc.vector.tensor_tensor(cond_rate[:], cat1[0:32, :], cr[:], ALU.add)
        cyc_fea = two_mlp(cycT[:], "cyc_fc1", "cyc_b1", "cyc_fc2", "cyc_b2")
        cc = gate_mod(cond_rate[:], cyc_fea[:], "ec_fc1", "ec_fc2", "ec_fcb",
                      "ec_g1", "ec_g2", "ec_gb", "ec")
        cm1 = small.tile([1, CL], f32, tag="cm1")
        nc.scalar.activation(cm1[:], cycT[:], AF.Identity, bias=cb(1, 2), scale=1.0)
        psd = pp.tile([32, CL], f32, tag="fc1_0")
        nc.tensor.matmul(psd[:], tw["dNw"][:], cm1[:], start=True, stop=True)
        ccd = small.tile([32, CL], f32, tag="ccd")
        nc.vector.tensor_tensor(ccd[:], cc[:], psd[:], ALU.mult)
        cond_cycle = small.tile([32, CL], f32, tag="cond_cycle")
        nc.vector.tensor_tensor(cond_cycle[:], cond_rate[:], ccd[:], ALU.add)
        if DBG:
            nc.sync.dma_start(d_dbg["condc"][:], cond_cycle[:])

        ps = pp.tile([64, CL], f32, tag="fc1_0")
        nc.tensor.matmul(ps[:], tw["v_fc1"][:], u0[:], start=True, stop=True)
        h1 = small.tile([64, CL], f32, tag="h1")
        nc.scalar.activation(h1[:], ps[:], AF.Relu, bias=tw["v_b1"][:], scale=1.0)
        ps = pp.tile([64, CL], f32, tag="fc1_0")
        nc.tensor.matmul(ps[0:32, :], tw["v_fc2"][:], h1[:], start=True, stop=True)
        xvol = small.tile([32, CL], f32, tag="xvol")
        nc.scalar.activation(xvol[:], ps[0:32, :], AF.Identity, bias=tw["v_b2"][:],
                             scale=1.0)
        z = small.tile([32, CL], f32, tag="z")
        nc.vector.tensor_tensor(z[:], xvol[:], cond_cycle[:], ALU.add)
        ps = pp.tile([64, CL], f32, tag="fc1_0")
        nc.tensor.matmul(ps[0:32, :], tw["av_fc"][:], z[:], start=True, stop=True)
        av = small.tile([32, CL], f32, tag="av")
        nc.scalar.activation(av[:], ps[0:32, :], AF.Identity, bias=tw["av_b"][:],
                             scale=1.0)
        x = small.tile([32, CL], f32, tag="x")
        softplus(x[:], av[:], 32, 'spx')
        ps = pp.tile([64, CL], f32, tag="fc1_0")
        nc.tensor.matmul(ps[0:32, :], tw["q_fc1"][:], x[:], start=True, stop=True)
        g1 = small.tile([32, CL], f32, tag="g1")
        nc.scalar.activation(g1[:], ps[0:32, :], AF.Identity, bias=tw["q_b1"][:],
                             scale=1.0)
        y1 = small.tile([32, CL], f32, tag="y1")
        softplus(y1[:], g1[:], 32, 'spy')
        ps = pp.tile([64, CL], f32, tag="fc1_0")
        nc.tensor.matmul(ps[0:1, :], tw["q_fc2"][:], y1[:], start=True, stop=True)
        g2 = small.tile([1, CL], f32, tag="g2")
        nc.scalar.activation(g2[:], ps[0:1, :], AF.Identity, bias=tw["q_b2"][:],
                             scale=1.0)
        if DBG:
            nc.sync.dma_start(d_dbg["g2d"][:], g2[:])
        qout = small.tile([1, CL], f32, tag="qout")
        softplus(qout[:], g2[:], 1, 'spq')

        s2 = small.tile([1, CL], f32, tag="s2")
        nc.scalar.activation(s2[:], g2[:], AF.Sigmoid, bias=cb(1, 0), scale=1.0)
        ps = pp.tile([64, CL], f32, tag="fc1_0")
        nc.tensor.matmul(ps[0:32, :], tw["q_fc2T"][:], s2[:], start=True, stop=True)
        sg1 = small.tile([32, CL], f32, tag="sg1")
        nc.scalar.activation(sg1[:], g1[:], AF.Sigmoid, bias=cb(32, 0), scale=1.0)
        dg1 = small.tile([32, CL], f32, tag="dg1")
        nc.vector.tensor_tensor(dg1[:], ps[0:32, :], sg1[:], ALU.mult)
        ps = pp.tile([64, CL], f32, tag="fc1_0")
        nc.tensor.matmul(ps[0:32, :], tw["q_fc1T"][:], dg1[:], start=True, stop=True)
        sa = small.tile([32, CL], f32, tag="sa")
        nc.scalar.activation(sa[:], av[:], AF.Sigmoid, bias=cb(32, 0), scale=1.0)
        da = small.tile([32, CL], f32, tag="da")
        nc.vector.tensor_tensor(da[:], ps[0:32, :], sa[:], ALU.mult)
        ps = pp.tile([64, CL], f32, tag="fc1_0")
        nc.tensor.matmul(ps[0:32, :], tw["av_fcT"][:], da[:], start=True, stop=True)
        dz = small.tile([32, CL], f32, tag="dz")
        nc.vector.tensor_copy(dz[:], ps[0:32, :])
        ps = pp.tile([64, CL], f32, tag="fc1_0")
        nc.tensor.matmul(ps[:], tw["v_fc2T"][:], dz[:], start=True, stop=True)
        hmask = small.tile([64, CL], f32, tag="hmask")
        nc.vector.tensor_scalar(out=hmask[:], in0=h1[:], scalar1=0.0,
                                scalar2=None, op0=ALU.is_gt)
        dh1 = small.tile([64, CL], f32, tag="dh1")
        nc.vector.tensor_tensor(dh1[:], ps[:], hmask[:], ALU.mult)
        ps = pp.tile([64, CL], f32, tag="fc1_0")
        nc.tensor.matmul(ps[0:3, :], tw["v_fc1T"][:], dh1[:], start=True, stop=True)
        dvii = small.tile([3, CL], f32, tag="dvii")
        nc.vector.tensor_copy(dvii[:], ps[0:3, :])

        nc.sync.dma_start(d_outq[:], qout[:])
        nc.sync.dma_start(d_outg[:], dvii[2:3, :])

    nc.compile()
    return nc


def _stack_b2m(v):
    return np.ascontiguousarray(np.tile(v, 4).reshape(128, 1).astype(np.float32))


def _stack_rep_edges(v):
    # [EL] -> [128, NG, 512]; row 32c+f, (g, j) = v[g*2048 + c*512 + j]
    v = v.reshape(NG, NS, 512)
    out = np.empty((128, NG, 512), np.float32)
    for c in range(NS):
        out[32 * c:32 * (c + 1)] = v[:, c, :][None, :, :]
    return np.ascontiguousarray(out)


def _stack_rep_nodes(v):
    # [NL] -> [128, 512]
    v = v.reshape(NS, 512)
    out = np.empty((128, 512), np.float32)
    for c in range(NS):
        out[32 * c:32 * (c + 1)] = v[c][None, :]
    return np.ascontiguousarray(out)


def _k2rep(w2col):
    o = np.empty((128, 2, 32), np.float32)
    o[:, 0, :] = w2col[0:128, None]
    o[:, 1, :] = w2col[128:256, None]
    return np.ascontiguousarray(o)


def _ktiles(W):  # [256, 32] -> [128, 2, 32]
    return np.ascontiguousarray(np.stack([W[0:128], W[128:256]], axis=1))


def _b4(b):  # [512] -> [128, 4]
    return np.ascontiguousarray(b.reshape(4, 128).T)


def _prep_inputs(inputs):
    ew = _a(inputs["elem_weights"])
    ef = _a(inputs["elem_fea"])
    self_idx = np.asarray(inputs["self_fea_idx"])
    nbr_idx = np.asarray(inputs["nbr_fea_idx"])
    cry_idx = np.asarray(inputs["cry_elem_idx"])
    base = np.repeat(np.arange(C, dtype=np.int64) * NPC, NPC * NPC)
    i = np.tile(np.repeat(np.arange(NPC, dtype=np.int64), NPC), C)
    j = np.tile(np.tile(np.arange(NPC, dtype=np.int64), NPC), C)
    assert np.array_equal(self_idx, base + i), "unexpected self_fea_idx"
    assert np.array_equal(nbr_idx, base + j), "unexpected nbr_fea_idx"
    assert np.array_equal(cry_idx, np.repeat(np.arange(C, dtype=np.int64), NPC))

    p = inputs["params"]
    V = _a(inputs["V_window"])
    rate = _a(inputs["rate"])
    cyc = _a(inputs["cycle"])
    vii = _a(inputs["Vii"])

    shared = {}
    wemb = _a(p["embedding"]["w"])
    shared["wemb0"] = wemb[0:128]
    shared["wemb1"] = np.ascontiguousarray(wemb[128:200])
    shared["bemb"] = _a(p["embedding"]["b"]).reshape(31, 1)
    for l in range(3):
        for h in range(3):
            hp = p["graphs"][l]["heads"][h]
            W1g = _a(hp["gate"]["layers"][0]["w"])
            W1m = _a(hp["msg"]["layers"][0]["w"])
            W1full = np.concatenate([W1g, W1m], axis=1)  # [64, 512]
            shared[f"W1s_{l}{h}"] = np.ascontiguousarray(W1full[0:32])
            shared[f"W1n_{l}{h}"] = np.ascontiguousarray(W1full[32:64])
            shared[f"b1_{l}{h}"] = _b4(np.concatenate(
                [_a(hp["gate"]["layers"][0]["b"]), _a(hp["msg"]["layers"][0]["b"])]))
            shared[f"w2g_{l}{h}"] = _k2rep(_a(hp["gate"]["out"]["w"])[:, 0])
            shared[f"W2m_{l}{h}"] = _ktiles(_a(hp["msg"]["out"]["w"]))
            shared[f"b2m_{l}{h}"] = _stack_b2m(_a(hp["msg"]["out"]["b"]))
    for h in range(3):
        hp = p["cry_pool"][h]
        shared[f"W1c_{h}"] = np.ascontiguousarray(np.concatenate(
            [_a(hp["gate"]["layers"][0]["w"]), _a(hp["msg"]["layers"][0]["w"])],
            axis=1))
        shared[f"b1c_{h}"] = _b4(np.concatenate(
            [_a(hp["gate"]["layers"][0]["b"]), _a(hp["msg"]["layers"][0]["b"])]))
        shared[f"w2gc_{h}"] = _k2rep(_a(hp["gate"]["out"]["w"])[:, 0])
        shared[f"W2mc_{h}"] = _ktiles(_a(hp["msg"]["out"]["w"]))
        shared[f"b2mc_{h}"] = _stack_b2m(_a(hp["msg"]["out"]["b"]))
    shared.update({
        "rate_fc1": _a(p["rate_emb"]["fc1"]["w"]),
        "rate_b1": _a(p["rate_emb"]["fc1"]["b"]).reshape(-1, 1),
        "rate_fc2": _pad_cols(_a(p["rate_emb"]["fc2"]["w"]), 32),
        "rate_b2": _pad_rows(_a(p["rate_emb"]["fc2"]["b"]).reshape(-1, 1), 32),
        "cyc_fc1": _a(p["cycle_emb"]["fc1"]["w"]),
        "cyc_b1": _a(p["cycle_emb"]["fc1"]["b"]).reshape(-1, 1),
        "cyc_fc2": _pad_cols(_a(p["cycle_emb"]["fc2"]["w"]), 32),
        "cyc_b2": _pad_rows(_a(p["cycle_emb"]["fc2"]["b"]).reshape(-1, 1), 32),
        "er_fc1": _a(p["encode_rate"]["fc"]["w"])[0:32],
        "er_fc2": _a(p["encode_rate"]["fc"]["w"])[32:48],
        "er_fcb": _a(p["encode_rate"]["fc"]["b"]).reshape(-1, 1),
        "er_g1": _a(p["encode_rate"]["gate"]["w"])[0:32],
        "er_g2": _a(p["encode_rate"]["gate"]["w"])[32:48],
        "er_gb": _a(p["encode_rate"]["gate"]["b"]).reshape(-1, 1),
        "ec_fc1": _a(p["encode_cycle"]["fc"]["w"])[0:32],
        "ec_fc2": _a(p["encode_cycle"]["fc"]["w"])[32:48],
        "ec_fcb": _a(p["encode_cycle"]["fc"]["b"]).reshape(-1, 1),
        "ec_g1": _a(p["encode_cycle"]["gate"]["w"])[0:32],
        "ec_g2": _a(p["encode_cycle"]["gate"]["w"])[32:48],
        "ec_gb": _a(p["encode_cycle"]["gate"]["b"]).reshape(-1, 1),
        "dNw": _a(p["delta_N_w"]),
        "v_fc1": _a(p["enc_vol"]["fc1"]["w"]),
        "v_b1": _a(p["enc_vol"]["fc1"]["b"]).reshape(-1, 1),
        "v_fc2": _a(p["enc_vol"]["fc2"]["w"]),
        "v_b2": _a(p["enc_vol"]["fc2"]["b"]).reshape(-1, 1),
        "av_fc": _a(p["add_vol"]["fc"]["w"]),
        "av_b": _a(p["add_vol"]["fc"]["b"]).reshape(-1, 1),
        "q_fc1": _a(p["fc"]["fc1"]["w"]),
        "q_b1": _a(p["fc"]["fc1"]["b"]).reshape(-1, 1),
        "q_fc2": _a(p["fc"]["fc2"]["w"]),
        "q_b2": _a(p["fc"]["fc2"]["b"]).reshape(-1, 1),
        "v_fc1T": np.ascontiguousarray(_a(p["enc_vol"]["fc1"]["w"]).T),
        "v_fc2T": np.ascontiguousarray(_a(p["enc_vol"]["fc2"]["w"]).T),
        "av_fcT": np.ascontiguousarray(_a(p["add_vol"]["fc"]["w"]).T),
        "q_fc1T": np.ascontiguousarray(_a(p["fc"]["fc1"]["w"]).T),
        "q_fc2T": np.ascontiguousarray(_a(p["fc"]["fc2"]["w"]).T),
    })

    in_maps = []
    for k in range(NCORES):
        nsl = slice(k * NL, (k + 1) * NL)
        csl = slice(k * CL, (k + 1) * CL)
        ew_l = ew[nsl, 0]
        m = dict(shared)
        m["efT"] = np.ascontiguousarray(ef[nsl].T)
        m["ew_row"] = np.ascontiguousarray(ew_l.reshape(1, NL))
        nbr_l = nbr_idx[k * EL:(k + 1) * EL] - k * NL
        m["wnbr_st"] = _stack_rep_edges(ew_l[nbr_l])
        m["wcry_st"] = _stack_rep_nodes(ew_l)
        m["u0T"] = np.ascontiguousarray(np.concatenate([V[csl], vii[csl]], 1).T)
        m["rateT"] = np.ascontiguousarray(rate[csl].T)
        m["cycT"] = np.ascontiguousarray(cyc[csl].T)
        in_maps.append(m)
    return in_maps


def kernel(**inputs) -> np.ndarray:
    from concourse.bass_utils import run_bass_kernel_spmd

    in_maps = _prep_inputs(inputs)
    if "nc" not in _CACHE:
        _CACHE["nc"] = _build()
    nc = _CACHE["nc"]
    res = run_bass_kernel_spmd(nc, in_maps, core_ids=list(range(NCORES)))
    q = np.concatenate([res.results[k]["outq"][0] for k in range(NCORES)])
    g = np.concatenate([res.results[k]["outg"][0] for k in range(NCORES)])
    return np.stack([q, g], axis=1).astype(np.float32)
